# revision 12
# baseline (speedup 1.0000x reference)
"""Trainium2 Bass kernel for the CGCNN model (8-core SPMD, graph-parallel).

Feature-major edge phase built on dma_gather(transpose=True):
- Shard graphs (64/core) -> contiguous node ranges via sorted batch vector.
- Local nodes relabeled by descending in-degree; local rows 0,1 and HNP are
  reserved (zeroA, poisonA, zeroB). Edge layout: partition p of block i owns
  dst node i*128+p, slot j holds its j-th incoming edge; position k within a
  chunk = slot*128 + p.
- Gather table split in two halves (A: local rows < HNP, B: >= HNP) so int16
  dma_gather indices stay < 8*HNP. Each 16-slot chunk issues two transposed
  gathers (G_A, G_B); positions whose src is in the other half point at that
  half's all-zero row, so PSUM-accumulating I@G_A + I@G_B merges them.
  Padded slots point at the poison row (f-half -30000 -> sigmoid == 0).
- Per 4-slot group: 2 ef matmuls (Wef_f/Wef_s vs eaT tiles) + 4 identity
  moves add G_A/G_B into a [128,1024] PSUM tile; tanh(x/2+bf/2), exp(x+bs)
  (bias via ACT bias column) write back into G_A; deferred per-chunk Ln +
  msg stt; aggregation = per-slot identity-matmul PSUM accumulation, already
  feature-major; invc scale + residual add into h_conv.
- BatchNorm: tail-only mask, reduces (squares staged bf16), AllReduce,
  fused scale/bias+ReLU, residual. Pooling via one-hot matmuls + tiny MLP.
"""
import os
import sys
import numpy as np

sys.path.insert(0, '/opt/trn_rl_repo')
os.environ.setdefault("NEURON_SCRATCHPAD_PAGE_SIZE", "256")

import ml_dtypes

BF16NP = ml_dtypes.bfloat16

N = 50000
E = 1600000
HID = 128
NGRAPH = 512
NCONV = 4
EDGE_DIM = 64
NCORES = 8
GPC = NGRAPH // NCORES
BN_EPS = 1e-5
POISON_VAL = -30000.0
CW = 16        # slots per chunk (chunk = CW*128 gather positions)

_CACHE = {}


def _host_prep(inputs):
    x = np.asarray(inputs['x'], np.float32)
    ei = np.asarray(inputs['edge_index']).astype(np.int64)
    ea = np.asarray(inputs['edge_attr'], np.float32)
    batch = np.asarray(inputs['batch']).astype(np.int64)
    src, dst = ei[0], ei[1]

    deg = np.bincount(dst, minlength=N)
    node_start = np.searchsorted(batch, np.arange(0, NGRAPH + 1, GPC))
    n_c = np.diff(node_start)
    NPAD = int(np.ceil((n_c.max() + 5) / 256.0) * 256)
    NB = NPAD // 128
    HNP = NPAD // 2
    # reserved local rows on every core: 0 zeroA, 1 poisonA, HNP zeroB
    free_rows = np.concatenate([np.arange(2, HNP), np.arange(HNP + 1, NPAD)])

    percore = []
    rows_of = np.empty(N, np.int64)   # global padded row: c*NPAD + r
    for c in range(NCORES):
        ids = np.arange(node_start[c], node_start[c + 1])
        order = ids[np.argsort(-deg[ids], kind='stable')]
        percore.append(order)
        rows_of[order] = c * NPAD + free_rows[:len(order)]

    # common per-block max degree
    B = np.zeros((NCORES, NB), np.int64)
    for c in range(NCORES):
        d = np.zeros(NPAD, np.int64)
        lr = rows_of[percore[c]] - c * NPAD
        d[lr] = deg[percore[c]]
        B[c] = d.reshape(NB, 128).max(1)
    Bi = B.max(0)
    Bi = np.maximum(2, ((Bi + 1) // 2) * 2)
    S = int(Bi.sum())
    slot0 = np.concatenate([[0], np.cumsum(Bi)]).astype(np.int64)

    # edge slot assignment; secondary sort by src row for HBM locality
    ro = rows_of[dst]
    sr = rows_of[src]
    order_e = np.lexsort((sr, ro))
    ro_s = ro[order_e]
    src_s = src[order_e]
    e_s = order_e
    row_change = np.concatenate([[True], ro_s[1:] != ro_s[:-1]])
    row_first = np.where(row_change)[0]
    starts = np.repeat(row_first, np.diff(np.concatenate([row_first, [len(ro_s)]])))
    rank = np.arange(len(ro_s)) - starts
    c_e = ro_s // NPAD
    r_loc = ro_s % NPAD
    blk = r_loc // 128
    p_e = r_loc % 128
    slot = slot0[blk] + rank
    assert (rank < Bi[blk]).all()

    # chunk list (block, slot_lo, n_slots)
    chunk_slots = []
    for i in range(NB):
        c0 = 0
        while c0 < Bi[i]:
            w = min(CW, int(Bi[i]) - c0)
            chunk_slots.append((i, int(slot0[i]) + c0, w))
            c0 += w

    NPOS = S * 128
    eaT = np.zeros((NCORES, 64, NPOS), BF16NP)
    gixA = np.zeros((NCORES, 128, NPOS // 16), np.int16)
    gixB = np.zeros((NCORES, 128, NPOS // 16), np.int16)
    ea_bf = ea.astype(BF16NP)

    for c in range(NCORES):
        m = c_e == c
        pos = slot[m] * 128 + p_e[m]
        gsr = rows_of[src_s[m]]
        scc = gsr // NPAD
        srl = gsr % NPAD
        inA = srl < HNP
        pA = np.zeros(NPOS, np.int64)
        pB = np.zeros(NPOS, np.int64)
        pA[pos[inA]] = scc[inA] * HNP + srl[inA]
        pB[pos[~inA]] = scc[~inA] * HNP + (srl[~inA] - HNP)
        cov = np.zeros(NPOS, bool)
        cov[pos] = True
        pA[~cov] = 1                        # padding -> poisonA
        assert pA.max() < 32768 and pB.max() < 32768
        for arr, gx in ((pA.astype(np.int16), gixA), (pB.astype(np.int16), gixB)):
            for (_, s_lo, w) in chunk_slots:
                p0, n = s_lo * 128, w * 128
                wv = arr[p0:p0 + n].reshape(n // 16, 16).T
                gx[c][:, p0 // 16:(p0 + n) // 16] = np.tile(wv, (8, 1))
        eaT[c][:, pos] = ea_bf[e_s[m]].T

    # per-core host tensors
    invcF = np.zeros((NCORES, 128, NPAD), np.float32)
    K = int(max(1, np.ceil((NPAD - (n_c.min() + 3)) / 128.0)))
    onehot = np.zeros((NCORES, 128, NB * GPC), np.float32)
    tmask = np.zeros((NCORES, 128, K * 128), BF16NP)
    xT = np.zeros((NCORES, 9, NPAD), np.float32)
    for c in range(NCORES):
        lr = rows_of[percore[c]] - c * NPAD
        d = np.ones(NPAD, np.float32)
        d[lr] = np.maximum(deg[percore[c]], 1)
        invcF[c] = np.broadcast_to((0.5 / d)[None, :], (128, NPAD))
        g_loc = batch[percore[c]] - c * GPC
        gsz = np.bincount(g_loc, minlength=GPC).astype(np.float32)
        oh = np.zeros((NPAD, GPC), np.float32)
        oh[lr, g_loc] = 1.0 / np.maximum(gsz[g_loc], 1.0)
        onehot[c] = oh.reshape(NB, 128, GPC).transpose(1, 0, 2).reshape(128, NB * GPC)
        lo = (NB - K) * 128
        real = np.zeros(NPAD, np.float32)
        real[lr] = 1.0
        tmask[c] = np.broadcast_to(real[lo:][None, :], (128, K * 128)).astype(BF16NP)
        xT[c][:, lr] = x[percore[c]].T

    meta = dict(NPAD=NPAD, NB=NB, Bi=Bi.tolist(), S=S,
                slot0=slot0.tolist(), K=K, chunks=chunk_slots)

    Wf = np.asarray(inputs['Wf'], np.float32)
    Ws = np.asarray(inputs['Ws'], np.float32)
    bf_ = np.asarray(inputs['bf'], np.float32)
    bs_ = np.asarray(inputs['bs'], np.float32)
    Wsrc = np.concatenate([np.concatenate([Wf[l, HID:2 * HID], Ws[l, HID:2 * HID]], 1)
                           for l in range(NCONV)], 1)          # [128, 4*256]
    Wdst = np.concatenate([np.concatenate([Wf[l, :HID], Ws[l, :HID]], 1)
                           for l in range(NCONV)], 1)          # [128, 4*256]
    Wef = np.concatenate([np.concatenate([Wf[l, 2 * HID:], Ws[l, 2 * HID:]], 1)
                          for l in range(NCONV)], 1).astype(BF16NP)  # [64, 4*256]
    biasfh = (bf_.T / 2.0).copy()          # [128, 4] (pre-halved for tanh)
    biassc = bs_.T.copy()                  # [128, 4]
    poison = np.zeros((1, 256), BF16NP)
    poison[0, :HID] = POISON_VAL
    zrow = np.zeros((1, 256), BF16NP)

    common = dict(
        W_emb=np.asarray(inputs['W_emb'], np.float32),
        bemb_row=np.asarray(inputs['b_emb'], np.float32)[None, :],
        ones1=np.ones((1, 128), np.float32),
        Wsrc=Wsrc, Wdst=Wdst, Wef=Wef,
        biasfh=biasfh, biassc=biassc,
        gammaA=np.asarray(inputs['gamma'], np.float32).T.copy(),
        betaA=np.asarray(inputs['beta'], np.float32).T.copy(),
        W1=np.asarray(inputs['W1'], np.float32),
        b1=np.asarray(inputs['b1'], np.float32)[:, None],
        W2=np.pad(np.asarray(inputs['W2'], np.float32), ((0, 64), (0, 0))),
        b2=np.asarray(inputs['b2'], np.float32)[:, None],
        I128b=np.eye(128, dtype=BF16NP),
        I128f=np.eye(128, dtype=np.float32),
        poison=poison, zrow=zrow,
    )
    in_maps = []
    for c in range(NCORES):
        m = dict(common)
        m.update(xT=xT[c], eaT=eaT[c], gixA=gixA[c], gixB=gixB[c],
                 invcF=invcF[c], onehot=onehot[c], tmask=tmask[c])
        in_maps.append(m)
    return meta, in_maps, percore, n_c


def _build(meta):
    import concourse.bass as bass
    import concourse.bacc as bacc
    import concourse.tile as tile
    from concourse import mybir

    F32 = mybir.dt.float32
    F16 = mybir.dt.float16
    BF = mybir.dt.bfloat16
    I16 = mybir.dt.int16
    AF = mybir.ActivationFunctionType
    OP = mybir.AluOpType

    NPAD, NB, Bi, S = meta['NPAD'], meta['NB'], meta['Bi'], meta['S']
    K = meta['K']
    slot0 = meta['slot0']
    chunks = meta['chunks']
    HNP = NPAD // 2
    HB = HNP // 128
    NPOS = S * 128
    RG = [list(range(NCORES))]

    nc = bacc.Bacc("TRN2", target_bir_lowering=False, debug=False,
                   num_devices=NCORES)

    def P_(name, shape, dt):
        return nc.declare_dram_parameter(name, shape, dt, isOutput=False)

    xT_d = P_('xT', [9, NPAD], F32)
    eaT_d = P_('eaT', [64, NPOS], BF)
    gixA_d = P_('gixA', [128, NPOS // 16], I16)
    gixB_d = P_('gixB', [128, NPOS // 16], I16)
    invcF_d = P_('invcF', [128, NPAD], F32)
    onehot_d = P_('onehot', [128, NB * GPC], F32)
    tmask_d = P_('tmask', [128, K * 128], BF)
    Wemb_d = P_('W_emb', [9, 128], F32)
    bembr_d = P_('bemb_row', [1, 128], F32)
    ones1_d = P_('ones1', [1, 128], F32)
    Wsrc_d = P_('Wsrc', [128, NCONV * 256], F32)
    Wdst_d = P_('Wdst', [128, NCONV * 256], F32)
    Wef_d = P_('Wef', [64, NCONV * 256], BF)
    biasfh_d = P_('biasfh', [128, NCONV], F32)
    biassc_d = P_('biassc', [128, NCONV], F32)
    gammaA_d = P_('gammaA', [128, NCONV], F32)
    betaA_d = P_('betaA', [128, NCONV], F32)
    W1_d = P_('W1', [128, 64], F32)
    b1_d = P_('b1', [64, 1], F32)
    W2_d = P_('W2', [128, 1], F32)
    b2_d = P_('b2', [1, 1], F32)
    I128b_d = P_('I128b', [128, 128], BF)
    I128f_d = P_('I128f', [128, 128], F32)
    poison_d = P_('poison', [1, 256], BF)
    zrow_d = P_('zrow', [1, 256], BF)
    out_d = nc.declare_dram_parameter('outg', [1, GPC], F32, isOutput=True)

    with tile.TileContext(nc) as tc:
        with tc.tile_pool(name="res", bufs=1) as res, \
             tc.tile_pool(name="gp", bufs=3) as gp, \
             tc.tile_pool(name="wk", bufs=3) as wk, \
             tc.tile_pool(name="ea", bufs=2) as eap, \
             tc.tile_pool(name="ps", bufs=2, space="PSUM") as ps, \
             tc.tile_pool(name="psa", bufs=2, space="PSUM") as psa, \
             tc.tile_pool(name="pst", bufs=2, space="PSUM") as pstp, \
             tc.tile_pool(name="dram", bufs=1, space="DRAM") as dram:

            def load(shape, dt, d, tag):
                t = res.tile(shape, dt, tag=tag)
                nc.sync.dma_start(t[:], d[:])
                return t

            invcF_sb = load([128, NPAD], F32, invcF_d, 'invcF')
            tmask_sb = load([128, K * 128], BF, tmask_d, 'tmask')
            Wemb_sb = load([9, 128], F32, Wemb_d, 'wemb')
            bembr_sb = load([1, 128], F32, bembr_d, 'bembr')
            ones1_sb = load([1, 128], F32, ones1_d, 'ones1')
            Wsrc_sb = load([128, NCONV * 256], F32, Wsrc_d, 'wsrc')
            Wdst_sb = load([128, NCONV * 256], F32, Wdst_d, 'wdst')
            Wef_sb = load([64, NCONV * 256], BF, Wef_d, 'wef')
            biasfh_sb = load([128, NCONV], F32, biasfh_d, 'biasfh')
            biassc_sb = load([128, NCONV], F32, biassc_d, 'biassc')
            gammaA_sb = load([128, NCONV], F32, gammaA_d, 'gamA')
            betaA_sb = load([128, NCONV], F32, betaA_d, 'betA')
            W1_sb = load([128, 64], F32, W1_d, 'w1')
            b1_sb = load([64, 1], F32, b1_d, 'b1')
            W2_sb = load([128, 1], F32, W2_d, 'w2')
            b2_sb = load([1, 1], F32, b2_d, 'b2')
            I128b_sb = load([128, 128], BF, I128b_d, 'idb')
            I128f_sb = load([128, 128], F32, I128f_d, 'idf')
            poison_sb = load([1, 256], BF, poison_d, 'poi')
            zrow_sb = load([1, 256], BF, zrow_d, 'zrw')

            h_loc = res.tile([128, NPAD], F32, tag='hloc')
            h_conv = res.tile([128, NPAD], F32, tag='hconv')
            sqbuf = res.tile([128, NPAD], BF, tag='sqbuf')
            PdstT = res.tile([128, 2 * NPAD], BF, tag='pdstT')  # [f|s]
            stats_sb = res.tile([128, 2], F32, tag='stats')
            scrg = res.tile([1, 8], mybir.dt.int32, tag='scrg')

            tbl_inA = dram.tile([HNP, 256], BF, tag='tblinA')
            tbl_inB = dram.tile([HNP, 256], BF, tag='tblinB')
            tbl_shA = dram.tile([NCORES * HNP, 256], BF, tag='tblshA')
            tbl_shB = dram.tile([NCORES * HNP, 256], BF, tag='tblshB')
            stats_in = dram.tile([128, 2], F32, tag='stin')
            stats_out = dram.tile([128, 2], F32, tag='stout')

            zcol = res.tile([128, 1], F32, tag='zcol')
            nc.vector.memset(zcol[:], 0.0)
            nc.const_aps.aps[(F32, 0.0)] = zcol[:]
            ocol = res.tile([128, 1], F32, tag='ocol')
            nc.vector.memset(ocol[:], 1.0)
            nc.const_aps.aps[(F32, 1.0)] = ocol[:]

            # ---- embed ----
            for t in range(NB):
                xt = wk.tile([9, 128], F32, tag='xt')
                nc.sync.dma_start(xt[:], xT_d[:, t * 128:(t + 1) * 128])
                pe = ps.tile([128, 1024], F32, tag='eps')
                nc.tensor.matmul(pe[:, 0:128], lhsT=Wemb_sb[:], rhs=xt[:],
                                 start=True, stop=False)
                nc.tensor.matmul(pe[:, 0:128], lhsT=bembr_sb[:], rhs=ones1_sb[:],
                                 start=False, stop=True)
                nc.vector.tensor_copy(h_loc[:, t * 128:(t + 1) * 128],
                                      pe[:, 0:128])

            for l in range(NCONV):
                lc = slice(l * 256, (l + 1) * 256)
                # ---- node phase ----
                for t in range(NB):
                    hsl = h_loc[:, t * 128:(t + 1) * 128]
                    pn = ps.tile([128, 1024], F32, tag='eps')
                    nc.tensor.matmul(pn[:, 0:256], lhsT=hsl, rhs=Wsrc_sb[:, lc],
                                     start=True, stop=True)
                    nc.tensor.matmul(pn[:, 512:640],
                                     lhsT=Wdst_sb[:, l * 256:l * 256 + 128],
                                     rhs=hsl, start=True, stop=True)
                    nc.tensor.matmul(pn[:, 640:768],
                                     lhsT=Wdst_sb[:, l * 256 + 128:(l + 1) * 256],
                                     rhs=hsl, start=True, stop=True)
                    st = wk.tile([128, 256], BF, tag='tstage')
                    nc.vector.tensor_copy(st[:], pn[:, 0:256])
                    tin = tbl_inA if t < HB else tbl_inB
                    r0 = (t * 128) % HNP
                    nc.sync.dma_start(tin[r0:r0 + 128, :], st[:])
                    nc.vector.tensor_copy(PdstT[:, t * 128:(t + 1) * 128],
                                          pn[:, 512:640])
                    nc.vector.tensor_copy(
                        PdstT[:, NPAD + t * 128:NPAD + (t + 1) * 128],
                        pn[:, 640:768])
                # reserved rows: zeroA, poisonA, zeroB
                nc.sync.dma_start(tbl_inA[0:1, :], zrow_sb[:])
                nc.sync.dma_start(tbl_inA[1:2, :], poison_sb[:])
                nc.sync.dma_start(tbl_inB[0:1, :], zrow_sb[:])
                nc.gpsimd.collective_compute(
                    "AllGather", OP.bypass, replica_groups=RG,
                    ins=[tbl_inA.opt()], outs=[tbl_shA.opt()])
                nc.gpsimd.collective_compute(
                    "AllGather", OP.bypass, replica_groups=RG,
                    ins=[tbl_inB.opt()], outs=[tbl_shB.opt()])
                tprobe = wk.tile([1, 128], BF, tag='tprobe')
                nc.gpsimd.dma_start(tprobe[:], tbl_shA[0:1, 0:128])
                nc.gpsimd.tensor_copy(scrg[0:1, 0:1].bitcast(BF),
                                      tprobe[0:1, 0:2])
                tprobe2 = wk.tile([1, 128], BF, tag='tprobe2')
                nc.gpsimd.dma_start(tprobe2[:], tbl_shB[0:1, 0:128])
                nc.gpsimd.tensor_copy(scrg[0:1, 1:2].bitcast(BF),
                                      tprobe2[0:1, 0:2])

                # ---- edge phase ----
                state = dict(pend=[], pag=None)

                def flush():
                    pend = state['pend']
                    if not pend:
                        return
                    for (GA, cw, blkk, first, last) in pend:
                        sp_ap = GA[:, cw * 128:2 * cw * 128]
                        nc.scalar.activation(sp_ap, sp_ap, AF.Ln, bias=1.0)
                    for (GA, cw, blkk, first, last) in pend:
                        nc.vector.scalar_tensor_tensor(
                            out=GA[:, 0:cw * 128],
                            in0=GA[:, 0:cw * 128].bitcast(F16),
                            scalar=ocol[:, 0:1],
                            in1=GA[:, cw * 128:2 * cw * 128],
                            op0=OP.add, op1=OP.mult)
                    for (GA, cw, blkk, first, last) in pend:
                        pag = state['pag']
                        for j in range(cw):
                            nc.tensor.matmul(
                                pag[:], lhsT=I128b_sb[:],
                                rhs=GA[:, j * 128:(j + 1) * 128],
                                start=(first and j == 0),
                                stop=(last and j == cw - 1))
                        if last:
                            bs = slice(blkk * 128, (blkk + 1) * 128)
                            agn = wk.tile([128, 128], F32, tag='agn')
                            nc.vector.tensor_tensor(
                                out=agn[:], in0=pag[:], in1=invcF_sb[:, bs],
                                op=OP.mult)
                            nc.vector.tensor_tensor(
                                out=h_conv[:, bs], in0=agn[:],
                                in1=h_loc[:, bs], op=OP.add)
                    state['pend'] = []

                for (blkk, s_lo, w) in chunks:
                    first = (s_lo == slot0[blkk])
                    last = (s_lo + w == slot0[blkk] + Bi[blkk])
                    if first:
                        flush()
                        state['pag'] = psa.tile([128, 128], F32, tag='agg', name='pag')
                    p0 = s_lo * 128
                    npos = w * 128
                    GA = gp.tile([128, 2 * CW * 128], BF, tag='GA')
                    GB = gp.tile([128, 2 * CW * 128], BF, tag='GB')
                    gxa = wk.tile([128, CW * 8], I16, tag='gxa')
                    gxb = wk.tile([128, CW * 8], I16, tag='gxb')
                    nc.sync.dma_start(gxa[:, 0:npos // 16],
                                      gixA_d[:, p0 // 16:(p0 + npos) // 16])
                    nc.sync.dma_start(gxb[:, 0:npos // 16],
                                      gixB_d[:, p0 // 16:(p0 + npos) // 16])
                    nc.gpsimd.dma_gather(
                        out_ap=GA[:, 0:2 * npos].rearrange(
                            "p (c k) -> p c k", c=2),
                        in_ap=tbl_shA[:, :], idxs_ap=gxa[:, 0:npos // 16],
                        num_idxs=npos, num_idxs_reg=npos, elem_size=256,
                        transpose=True, single_packet=False)
                    nc.gpsimd.dma_gather(
                        out_ap=GB[:, 0:2 * npos].rearrange(
                            "p (c k) -> p c k", c=2),
                        in_ap=tbl_shB[:, :], idxs_ap=gxb[:, 0:npos // 16],
                        num_idxs=npos, num_idxs_reg=npos, elem_size=256,
                        transpose=True, single_packet=False)
                    ea_blk = eap.tile([64, CW * 128], BF, tag='eab')
                    nc.sync.dma_start(ea_blk[:, 0:npos], eaT_d[:, p0:p0 + npos])
                    # Pdst add into G_A (both halves)
                    bs = slice(blkk * 128, (blkk + 1) * 128)
                    pd2 = PdstT[:].rearrange("p (h n) -> p h n", h=2)
                    nc.vector.tensor_tensor(
                        out=GA[:, 0:2 * npos].rearrange(
                            "p (h s d) -> p h s d", h=2, d=128),
                        in0=GA[:, 0:2 * npos].rearrange(
                            "p (h s d) -> p h s d", h=2, d=128),
                        in1=pd2[:, :, bs].unsqueeze(2)
                        .to_broadcast([128, 2, w, 128]),
                        op=OP.add)
                    for g in range(0, w, 4):
                        gw = min(4, w - g)
                        g0 = g * 128
                        gn = gw * 128
                        pe = ps.tile([128, 1024], F32, tag='eps')
                        nc.tensor.matmul(pe[:, 0:gn],
                                         lhsT=Wef_sb[:, l * 256:l * 256 + 128],
                                         rhs=ea_blk[:, g0:g0 + gn],
                                         start=True, stop=False)
                        nc.tensor.matmul(pe[:, 512:512 + gn],
                                         lhsT=Wef_sb[:, l * 256 + 128:(l + 1) * 256],
                                         rhs=ea_blk[:, g0:g0 + gn],
                                         start=True, stop=False)
                        nc.tensor.matmul(pe[:, 0:gn], lhsT=I128b_sb[:],
                                         rhs=GA[:, g0:g0 + gn],
                                         start=False, stop=False)
                        nc.tensor.matmul(pe[:, 512:512 + gn], lhsT=I128b_sb[:],
                                         rhs=GA[:, npos + g0:npos + g0 + gn],
                                         start=False, stop=False)
                        nc.tensor.matmul(pe[:, 0:gn], lhsT=I128b_sb[:],
                                         rhs=GB[:, g0:g0 + gn],
                                         start=False, stop=True)
                        nc.tensor.matmul(pe[:, 512:512 + gn], lhsT=I128b_sb[:],
                                         rhs=GB[:, npos + g0:npos + g0 + gn],
                                         start=False, stop=True)
                        # th = tanh(f/2 + bf/2) (F16, over GA f-half)
                        nc.scalar.activation(
                            GA[:, g0:g0 + gn].bitcast(F16),
                            pe[:, 0:gn], AF.Tanh, scale=0.5,
                            bias=biasfh_sb[:, l:l + 1])
                        # e = exp(s + bs) (bf16, GA s-half)
                        nc.scalar.activation(
                            GA[:, npos + g0:npos + g0 + gn],
                            pe[:, 512:512 + gn], AF.Exp,
                            bias=biassc_sb[:, l:l + 1])
                    state['pend'].append((GA, w, blkk, first, last))
                    if len(state['pend']) >= 2:
                        flush()
                flush()

                # ---- batch norm ----
                tlo = (NB - K) * 128
                nc.vector.scalar_tensor_tensor(
                    out=h_conv[:, tlo:NPAD], in0=h_conv[:, tlo:NPAD],
                    scalar=1.0, in1=tmask_sb[:], op0=OP.mult, op1=OP.mult)
                nc.vector.tensor_reduce(stats_sb[:, 0:1], h_conv[:],
                                        axis=mybir.AxisListType.X, op=OP.add)
                nc.vector.scalar_tensor_tensor(
                    out=sqbuf[:], in0=h_conv[:], scalar=1.0, in1=h_conv[:],
                    op0=OP.mult, op1=OP.mult)
                nc.vector.tensor_reduce(stats_sb[:, 1:2], sqbuf[:],
                                        axis=mybir.AxisListType.X, op=OP.add)
                nc.sync.dma_start(stats_in[:], stats_sb[:])
                nc.gpsimd.collective_compute(
                    "AllReduce", OP.add, replica_groups=RG,
                    ins=[stats_in.opt()], outs=[stats_out.opt()])
                gst = wk.tile([128, 2], F32, tag='gst')
                nc.sync.dma_start(gst[:], stats_out[:])
                mu = wk.tile([128, 8], F32, tag='mu')
                nc.vector.tensor_scalar_mul(mu[:, 0:1], gst[:, 0:1], 1.0 / N)
                nc.vector.tensor_scalar_mul(mu[:, 1:2], gst[:, 1:2], 1.0 / N)
                nc.vector.tensor_tensor(out=mu[:, 2:3], in0=mu[:, 0:1],
                                        in1=mu[:, 0:1], op=OP.mult)
                nc.vector.tensor_tensor(out=mu[:, 3:4], in0=mu[:, 1:2],
                                        in1=mu[:, 2:3], op=OP.subtract)
                nc.vector.tensor_scalar_add(mu[:, 3:4], mu[:, 3:4], BN_EPS)
                nc.scalar.activation(mu[:, 4:5], mu[:, 3:4], AF.Ln)
                nc.scalar.activation(mu[:, 5:6], mu[:, 4:5], AF.Exp,
                                     scale=-0.5)
                nc.vector.tensor_tensor(out=mu[:, 6:7], in0=gammaA_sb[:, l:l + 1],
                                        in1=mu[:, 5:6], op=OP.mult)
                nc.vector.scalar_tensor_tensor(
                    out=mu[:, 7:8], in0=mu[:, 0:1], scalar=mu[:, 6:7],
                    op0=OP.mult, op1=OP.subtract, in1=betaA_sb[:, l:l + 1])
                nc.vector.tensor_scalar_mul(mu[:, 7:8], mu[:, 7:8], -1.0)
                nc.scalar.activation(h_conv[:], h_conv[:], AF.Relu,
                                     bias=mu[:, 7:8], scale=mu[:, 6:7])
                nc.vector.tensor_tensor(out=h_loc[:], in0=h_conv[:],
                                        in1=h_loc[:], op=OP.add)

            # ---- pooling + MLP ----
            ppool = psa.tile([128, GPC], F32, tag='pool', bufs=1)
            for i in range(NB):
                ptr = pstp.tile([128, 128], F32, tag='tr', bufs=1)
                nc.tensor.transpose(ptr[:], h_loc[:, i * 128:(i + 1) * 128],
                                    I128f_sb[:])
                hn = wk.tile([128, 128], F32, tag='hn')
                nc.vector.tensor_copy(hn[:], ptr[:])
                oht = wk.tile([128, GPC], F32, tag='oht')
                nc.sync.dma_start(oht[:], onehot_d[:, i * GPC:(i + 1) * GPC])
                nc.tensor.matmul(ppool[:], lhsT=hn[:], rhs=oht[:],
                                 start=(i == 0), stop=(i == NB - 1))
            pooled = wk.tile([128, GPC], F32, tag='pooled')
            nc.vector.tensor_copy(pooled[:], ppool[:])
            pz = ps.tile([128, 1024], F32, tag='eps')
            nc.tensor.matmul(pz[0:64, 0:GPC], lhsT=W1_sb[:], rhs=pooled[:],
                             start=True, stop=True)
            z1 = wk.tile([64, GPC], F32, tag='z1')
            nc.scalar.activation(z1[:], pz[0:64, 0:GPC], AF.Exp,
                                 bias=b1_sb[:, 0:1])
            nc.scalar.activation(z1[:], z1[:], AF.Ln, bias=1.0)
            pz2 = pstp.tile([128, 128], F32, tag='tr', bufs=1)
            nc.tensor.matmul(pz2[0:1, 0:GPC], lhsT=W2_sb[0:64, :], rhs=z1[:],
                             start=True, stop=True)
            zo = wk.tile([1, GPC], F32, tag='zo')
            nc.scalar.activation(zo[:], pz2[0:1, 0:GPC], AF.Identity,
                                 bias=b2_sb[0:1, 0:1])
            nc.sync.dma_start(out_d[:], zo[:])

    nc.compile()
    return nc


TRACE = False
LAST_RESULTS = None


def kernel(**inputs):
    global LAST_RESULTS
    from concourse.bass_utils import run_bass_kernel_spmd

    meta, in_maps, percore, n_c = _host_prep(inputs)
    key = (meta['NPAD'], meta['S'], meta['K'], tuple(meta['Bi']))
    if key not in _CACHE:
        _CACHE[key] = _build(meta)
    nc = _CACHE[key]
    res = run_bass_kernel_spmd(nc, in_maps, list(range(NCORES)), trace=TRACE)
    LAST_RESULTS = res
    out = np.concatenate([np.asarray(res.results[c]['outg']).reshape(GPC)
                          for c in range(NCORES)])
    return out.astype(np.float32)


def bench(inputs, reps=8):
    """Steady-state device timing: jit once, inputs device-resident, time
    repeated executes (async-pipelined, block at end)."""
    import time
    import jax
    from jax.sharding import Mesh, PartitionSpec
    from jax.experimental.shard_map import shard_map
    from concourse import bass2jax
    from concourse.bass2jax import _bass_exec_p, partition_id_tensor, \
        install_neuronx_cc_hook
    from concourse import mybir

    meta, in_maps, percore, n_c = _host_prep(inputs)
    key = (meta['NPAD'], meta['S'], meta['K'], tuple(meta['Bi']))
    if key not in _CACHE:
        _CACHE[key] = _build(meta)
    nc = _CACHE[key]
    install_neuronx_cc_hook()
    n_cores = NCORES
    in_names, out_names, out_avals, zero_outs = [], [], [], []
    for alloc in nc.m.functions[0].allocations:
        if not isinstance(alloc, mybir.MemoryLocationSet):
            continue
        name = alloc.memorylocations[0].name
        pn = nc.partition_id_tensor.name if nc.partition_id_tensor else None
        if alloc.kind == "ExternalInput":
            if name != pn:
                in_names.append(name)
        elif alloc.kind == "ExternalOutput":
            out_names.append(name)
            shape = tuple(alloc.tensor_shape)
            dtype = mybir.dt.np(alloc.dtype)
            out_avals.append(jax.core.ShapedArray(shape, dtype))
            zero_outs.append(np.zeros(shape, dtype))
    n_params = len(in_names)
    n_outs = len(out_avals)
    all_names = list(in_names) + out_names
    pn = nc.partition_id_tensor.name if nc.partition_id_tensor else None
    if pn is not None:
        all_names.append(pn)

    def _body(*args):
        operands = list(args)
        if pn is not None:
            operands.append(partition_id_tensor())
        return tuple(_bass_exec_p.bind(
            *operands, out_avals=tuple(out_avals), in_names=tuple(all_names),
            out_names=tuple(out_names), lowering_input_output_aliases=(),
            sim_require_finite=True, sim_require_nnan=True, nc=nc))

    devices = jax.devices()[:n_cores]
    mesh = Mesh(np.asarray(devices), ("core",))
    in_specs = (PartitionSpec("core"),) * (n_params + n_outs)
    out_specs = (PartitionSpec("core"),) * len(out_names)
    sharded = jax.jit(shard_map(_body, mesh=mesh, in_specs=in_specs,
                                out_specs=out_specs, check_rep=False),
                      keep_unused=True)
    concat_in = [np.concatenate([np.asarray(in_maps[c][nm])
                                 for c in range(n_cores)], axis=0)
                 for nm in in_names]
    concat_zeros = [np.zeros((n_cores * z.shape[0], *z.shape[1:]), z.dtype)
                    for z in zero_outs]
    din = [jax.device_put(a) for a in concat_in]
    dzr = [jax.device_put(a) for a in concat_zeros]
    out = sharded(*din, *dzr)  # warmup + compile
    jax.block_until_ready(out)
    t0 = time.time()
    for _ in range(reps):
        out = sharded(*din, *dzr)
    jax.block_until_ready(out)
    dt = (time.time() - t0) / reps
    return dt, out


if __name__ == '__main__':
    import jax
    import reference as ref
    with jax.default_device(jax.devices('cpu')[0]):
        inputs = {k: np.asarray(v) for k, v in ref.setup_inputs().items()}
        exp = np.asarray(ref.reference(**ref.setup_inputs()))
    got = kernel(**inputs)
    rel = np.abs(got - exp) / np.maximum(np.abs(exp), 1e-6)
    print('rel err max/mean:', rel.max(), rel.mean())
